# revision 29
# baseline (speedup 1.0000x reference)
"""Bass/Trainium2 kernel for nn_BiasAttention (spatial attention over joints).

Reference computation (per batch b, time t):
  qkv = x @ Wqkv.T                          # [B,T,J,3C]
  q,k,v split into H=8 heads of hd=64
  attn = softmax(q k^T / sqrt(hd) + bias_h) # over joints J=17
  out  = (attn @ v) merged, @ Wproj.T + bproj

Sharding: data-parallel over B across 8 NeuronCores (4 batches each).

Per-core device pipeline (per batch, tokens = T*J = 4131):
  A. DMA-xbar transposed load of x (bf16)      -> xT  [c, tok]
  B. q/k projection (PE, W stationary)         -> qkT [d, tok]  (scale folded in Wq)
  C. v projection (PE, xT stationary)          -> v   [tok, d]  (natural layout)
  D. attention in groups of 7 timesteps (119 tokens):
       S^T   = k q^T          (PE, one matmul per (head, group))
       A     = exp(S^T) * Ebm (ACT exp, DVE mul; Ebm = exp(bias)+block-diag mask)
       O_un  = A^T.T @ v      (PE; extra N=1 matmul vs ones column -> row sums)
       r     = 1/sums         (DVE, batched per group)
       O     = O_un * r       (DVE tensor_scalar, fused PSUM->SBUF copy + bf16 cast)
  E. PE-transpose O -> oT [c, tok]
  F. projection + bias (PE, bias via K=1 matmul), fp32 out, DMA to DRAM
"""

import os
import numpy as np
import ml_dtypes

import concourse.bass as bass
import concourse.tile as tile
import concourse.mybir as mybir
from concourse import bacc

BF16 = np.float16  # 16-bit staging dtype (fp16: 10-bit mantissa, exp<=6 here so safe)

B, T, J, C, H = 32, 243, 17, 512, 8
HD = C // H                      # 64
NCORES = 8
B_LOC = B // NCORES              # 4 batches per core
TOK = T * J                      # 4131 tokens per batch
TOKP = 4144                      # padded to %16 for DMA-xbar transpose
TG = 7                           # timesteps per attention group
GW = TG * J                      # 119 tokens per full group
NG = (T + TG - 1) // TG          # 35 groups (34 full + one of 5 timesteps)
LASTW = (T - (NG - 1) * TG) * J  # 85 tokens in last group
CCH = C // 128                   # 4 contraction chunks
FP32 = mybir.dt.float32
BF = mybir.dt.float16


def _group_width(g):
    return GW if g < NG - 1 else LASTW


def _build_program():
    nc = bacc.Bacc("TRN2", num_devices=NCORES)

    xin = nc.dram_tensor("xin", [B_LOC, CCH, 128, TOK], BF, kind="ExternalInput").ap()
    wqk = nc.dram_tensor("wqk", [C, 2 * C], BF, kind="ExternalInput").ap()
    wv = nc.dram_tensor("wv", [C, C], BF, kind="ExternalInput").ap()
    wp = nc.dram_tensor("wp", [C, C], BF, kind="ExternalInput").ap()
    bp = nc.dram_tensor("bp", [1, C], BF, kind="ExternalInput").ap()
    ebm = nc.dram_tensor("ebm", [H, 128, GW], BF, kind="ExternalInput").ap()
    ident = nc.dram_tensor("ident", [128, 128], BF, kind="ExternalInput").ap()
    ones = nc.dram_tensor("ones", [128, 1], BF, kind="ExternalInput").ap()
    onesr = nc.dram_tensor("onesr", [1, 128], BF, kind="ExternalInput").ap()
    yout = nc.dram_tensor("yout", [B_LOC, TOK, C], FP32, kind="ExternalOutput").ap()

    with tile.TileContext(nc) as tc:
        with (
            tc.tile_pool(name="consts", bufs=1) as consts,
            tc.tile_pool(name="stage", bufs=1) as stage,
            tc.tile_pool(name="qkst", bufs=1) as qkst,
            tc.tile_pool(name="vst", bufs=1) as vst,
            tc.tile_pool(name="osb", bufs=3) as osbp,
            tc.tile_pool(name="absb", bufs=3) as absb,
            tc.tile_pool(name="rsb", bufs=2) as rsbp,
            tc.tile_pool(name="ysb", bufs=2) as ysbp,
            tc.tile_pool(name="ppb", bufs=2, space="PSUM") as ppb,
            tc.tile_pool(name="psc", bufs=2, space="PSUM") as psc,
            tc.tile_pool(name="po", bufs=2, space="PSUM") as po,
            tc.tile_pool(name="psum_s", bufs=2, space="PSUM") as psums_pool,
        ):
            # ---- persistent constants in SBUF ----
            w_sb = consts.tile([128, CCH, 2 * C], BF, name="w_sb")
            for cc in range(CCH):
                nc.sync.dma_start(w_sb[:, cc, :], wqk[128 * cc : 128 * (cc + 1), :])
            wv_sb = consts.tile([128, CCH, C], BF, name="wv_sb")
            for cc in range(CCH):
                nc.sync.dma_start(wv_sb[:, cc, :], wv[128 * cc : 128 * (cc + 1), :])
            wp_sb = consts.tile([128, CCH, C], BF, name="wp_sb")
            for cc in range(CCH):
                nc.sync.dma_start(wp_sb[:, cc, :], wp[128 * cc : 128 * (cc + 1), :])
            bp_sb = consts.tile([1, C], BF, name="bp_sb")
            nc.sync.dma_start(bp_sb[:, :], bp[:, :])
            ebm_sb = consts.tile([128, H, GW], BF, name="ebm_sb")
            for h in range(H):
                nc.sync.dma_start(ebm_sb[:, h, :], ebm[h, :, :])
            id_sb = consts.tile([128, 128], BF, name="id_sb")
            nc.sync.dma_start(id_sb[:, :], ident[:, :])
            ones_sb = consts.tile([128, 1], BF, name="ones_sb")
            nc.sync.dma_start(ones_sb[:, :], ones[:, :])
            onesr_sb = consts.tile([1, 128], BF, name="onesr_sb")
            nc.sync.dma_start(onesr_sb[:, :], onesr[:, :])

            for b in range(B_LOC):
                # ---- A: load pre-transposed x -> xT [c%128, (c//128, tok)] ----
                xT = stage.tile([128, CCH, TOK], BF, name="xT", tag="stg")
                for cc in range(CCH):
                    nc.sync.dma_start(xT[:, cc, :], xin[b, cc, :, :])

                # ---- B: q/k projection -> qkT [d%128, (d//128, tok)] ----
                qkT = qkst.tile([128, 2 * CCH, TOK], BF, name="qkT")
                ntk = (TOK + 511) // 512  # 9 chunks (last = 35)
                for dc in range(2 * CCH):
                    for tk in range(ntk):
                        t0 = 512 * tk
                        w = min(512, TOK - t0)
                        ps = ppb.tile([128, 512], FP32, name="ps_qk", tag="ppb")
                        for cc in range(CCH):
                            nc.tensor.matmul(
                                ps[:, :w],
                                lhsT=w_sb[:, cc, 128 * dc : 128 * (dc + 1)],
                                rhs=xT[:, cc, t0 : t0 + w],
                                start=(cc == 0),
                                stop=(cc == CCH - 1),
                            )
                        nc.scalar.copy(qkT[:, dc, t0 : t0 + w], ps[:, :w])

                # ---- C: v projection -> v_sb [tok_in_group, (group, d)] ----
                v_sb = vst.tile([128, NG, C], BF, name="v_sb")
                for g in range(NG):
                    wg = _group_width(g)
                    t0 = GW * g
                    ps = ppb.tile([128, 512], FP32, name="ps_v", tag="ppb")
                    for cc in range(CCH):
                        nc.tensor.matmul(
                            ps[:wg, :],
                            lhsT=xT[:, cc, t0 : t0 + wg],
                            rhs=wv_sb[:, cc, :],
                            start=(cc == 0),
                            stop=(cc == CCH - 1),
                        )
                    nc.vector.tensor_copy(v_sb[:wg, g, :], ps[:wg, :])

                # ---- D+E: attention per (group, head), then transpose ----
                oT = stage.tile([128, CCH, TOK], BF, name="oT", tag="stg2")
                for g in range(NG):
                    wg = _group_width(g)
                    t0 = GW * g
                    po_t = po.tile([128, 512], FP32, name="po_t", tag="po")
                    psums = psums_pool.tile([128, H], FP32, name="psums", tag="ps")
                    am_tiles = []
                    for h in range(H):
                        base = 64 * (h % 2)
                        qc = h // 2
                        kc = CCH + h // 2
                        pss = psc.tile([128, GW], FP32, name="pss", tag="psc")
                        nc.tensor.matmul(
                            pss[:wg, :wg],
                            lhsT=qkT[base : base + 64, kc, t0 : t0 + wg],
                            rhs=qkT[base : base + 64, qc, t0 : t0 + wg],
                            start=True,
                            stop=True,
                        )
                        ae = absb.tile([128, GW], BF, name="ae", tag="ab")
                        nc.scalar.activation(
                            ae[:wg, :wg],
                            pss[:wg, :wg],
                            mybir.ActivationFunctionType.Exp,
                        )
                        am = absb.tile([128, GW], BF, name="am", tag="ab")
                        nc.vector.tensor_mul(
                            am[:wg, :wg], ae[:wg, :wg], ebm_sb[:wg, h, :wg]
                        )
                        am_tiles.append(am)
                        # AV: O_un[(t,i), d] and row-sums into shared column bank
                        nc.tensor.matmul(
                            po_t[:wg, 64 * h : 64 * h + 64],
                            lhsT=am[:wg, :wg],
                            rhs=v_sb[:wg, g, 64 * h : 64 * h + 64],
                            start=True,
                            stop=True,
                        )
                        nc.tensor.matmul(
                            psums[:wg, h : h + 1],
                            lhsT=am[:wg, :wg],
                            rhs=ones_sb[:wg, :],
                            start=True,
                            stop=True,
                        )
                    r_sb = rsbp.tile([128, H], FP32, name="r_sb", tag="r")
                    nc.vector.reciprocal(r_sb[:wg, :], psums[:wg, :H])
                    o_sb = osbp.tile([128, C], BF, name="o_sb", tag="o")
                    for h in range(H):
                        nc.vector.tensor_scalar_mul(
                            o_sb[:wg, 64 * h : 64 * h + 64],
                            po_t[:wg, 64 * h : 64 * h + 64],
                            r_sb[:wg, h : h + 1],
                        )
                    # transpose O -> oT [c%128, (c//128, tok)]
                    for cc in range(CCH):
                        pt = psc.tile([128, GW], BF, name="pt", tag="psc")
                        nc.tensor.transpose(
                            pt[:, :wg],
                            o_sb[:wg, 128 * cc : 128 * (cc + 1)],
                            id_sb[:wg, :wg],
                        )
                        if cc % 2 == 0:
                            nc.scalar.copy(oT[:, cc, t0 : t0 + wg], pt[:, :wg])
                        else:
                            nc.vector.tensor_copy(oT[:, cc, t0 : t0 + wg], pt[:, :wg])

                # ---- F: output projection + bias, fp32 out ----
                ntk = (TOK + 127) // 128  # 33 chunks (last = 35)
                for tk in range(ntk):
                    t0 = 128 * tk
                    wt = min(128, TOK - t0)
                    psy = ppb.tile([128, 512], FP32, name="psy", tag="ppb")
                    for cc in range(CCH):
                        nc.tensor.matmul(
                            psy[:wt, :],
                            lhsT=oT[:, cc, t0 : t0 + wt],
                            rhs=wp_sb[:, cc, :],
                            start=(cc == 0),
                            stop=False,
                        )
                    # bias: out[m, n] += 1 * bp[n]  (K=1 matmul)
                    nc.tensor.matmul(
                        psy[:wt, :],
                        lhsT=onesr_sb[0:1, :wt],
                        rhs=bp_sb[:, :],
                        start=False,
                        stop=True,
                    )
                    y_sb = ysbp.tile([128, C], FP32, name="y_sb", tag="y")
                    nc.scalar.copy(y_sb[:wt, :], psy[:wt, :])
                    nc.sync.dma_start(yout[b, t0 : t0 + wt, :], y_sb[:wt, :])

    nc.compile()
    return nc


_RUNNER = None


def _get_runner():
    """Build the program once and wrap it in a cached jitted PJRT callable.

    Mirrors bass2jax.run_bass_via_pjrt's multi-core path, but caches the
    jitted function so repeated kernel() calls (and timing loops) don't
    re-trace/re-compile.
    """
    global _RUNNER
    if _RUNNER is not None:
        return _RUNNER

    import jax
    from jax.sharding import Mesh, PartitionSpec
    from jax.experimental.shard_map import shard_map
    from concourse.bass2jax import (
        _bass_exec_p,
        install_neuronx_cc_hook,
        partition_id_tensor,
    )

    install_neuronx_cc_hook()
    nc = _build_program()
    partition_name = (
        nc.partition_id_tensor.name if nc.partition_id_tensor else None
    )

    in_names, out_names, out_avals = [], [], []
    for alloc in nc.m.functions[0].allocations:
        if not isinstance(alloc, mybir.MemoryLocationSet):
            continue
        name = alloc.memorylocations[0].name
        if alloc.kind == "ExternalInput":
            if name != partition_name:
                in_names.append(name)
        elif alloc.kind == "ExternalOutput":
            out_names.append(name)
            out_avals.append(
                jax.core.ShapedArray(
                    tuple(alloc.tensor_shape), mybir.dt.np(alloc.dtype)
                )
            )
    n_outs = len(out_avals)
    dbg_zero = np.zeros((1, 2), np.uint32) if nc.dbg_addr is not None else None
    bind_names = in_names + out_names
    if partition_name is not None:
        bind_names = bind_names + [partition_name]

    def _body(*args):
        operands = list(args)
        if partition_name is not None:
            operands.append(partition_id_tensor())
        outs = _bass_exec_p.bind(
            *operands,
            out_avals=tuple(out_avals),
            in_names=tuple(bind_names),
            out_names=tuple(out_names),
            lowering_input_output_aliases=(),
            sim_require_finite=True,
            sim_require_nnan=True,
            nc=nc,
        )
        return tuple(outs)

    devices = jax.devices()[:NCORES]
    mesh = Mesh(np.asarray(devices), ("core",))
    nin_total = len(in_names) + n_outs
    sharded = jax.jit(
        shard_map(
            _body,
            mesh=mesh,
            in_specs=(PartitionSpec("core"),) * nin_total,
            out_specs=(PartitionSpec("core"),) * n_outs,
            check_rep=False,
        ),
        keep_unused=True,
    )
    _RUNNER = (sharded, in_names, out_names, out_avals, nc, dbg_zero)
    return _RUNNER


def _concat_inputs(in_maps):
    import jax
    from jax.sharding import Mesh, NamedSharding, PartitionSpec

    sharded, in_names, out_names, out_avals, nc, dbg_zero = _get_runner()
    mesh = Mesh(np.asarray(jax.devices()[:NCORES]), ("core",))
    sh = NamedSharding(mesh, PartitionSpec("core"))
    args = []
    dbg_name = nc.dbg_addr.name if nc.dbg_addr is not None else None
    for name in in_names:
        if name == dbg_name:
            args.append(np.concatenate([dbg_zero] * NCORES, axis=0))
        else:
            args.append(np.concatenate([m[name] for m in in_maps], axis=0))
    for av in out_avals:
        args.append(np.zeros((NCORES * av.shape[0], *av.shape[1:]), av.dtype))
    return [jax.device_put(a, sh) for a in args]


def _run_once(args):
    sharded = _get_runner()[0]
    return sharded(*args)


def _host_prep(x, Wqkv, Wproj, bproj, sd_emb, sd):
    """Host-side layout/dtype prep. Returns per-core input maps."""
    scale = HD ** -0.5
    Wq = Wqkv[:C] * scale              # fold 1/sqrt(hd) into Wq
    Wk = Wqkv[C : 2 * C]
    Wv = Wqkv[2 * C :]
    wqk = np.concatenate([Wq, Wk], axis=0).T.astype(BF16)   # [C, 2C] = WqkT
    wv = np.ascontiguousarray(Wv.T).astype(BF16)            # [C, C]
    wp = np.ascontiguousarray(Wproj.T).astype(BF16)         # [C, C]
    bp = bproj.reshape(1, C).astype(BF16)

    # bias gather: sd_emb[sd] -> [J, J, H] -> bias[h, i, j]
    bias = np.asarray(sd_emb)[np.asarray(sd)].transpose(2, 0, 1)  # [H, J, J]
    eb = np.exp(bias.astype(np.float64)).astype(np.float32)
    # S^T block layout is [row=(dt,j), col=(dt,i)] -> place exp(bias[i,j]) at [j,i]
    ebT = eb.transpose(0, 2, 1)
    ebm = np.zeros((H, 128, GW), dtype=np.float32)
    for dt in range(TG):
        r0 = J * dt
        ebm[:, r0 : r0 + J, r0 : r0 + J] = ebT
    ebm = ebm.astype(BF16)

    ident = np.eye(128, dtype=BF16)
    ones = np.ones((128, 1), dtype=BF16)
    onesr = np.ones((1, 128), dtype=BF16)

    x = np.asarray(x, dtype=np.float32).reshape(B, TOK, C).astype(BF16)
    # pre-transpose to [b, c//128, c%128, tok] so the device can DMA directly
    xt_all = np.ascontiguousarray(
        x.reshape(B, TOK, CCH, 128).transpose(0, 2, 3, 1)
    )
    in_maps = []
    for core in range(NCORES):
        xp = xt_all[core * B_LOC : (core + 1) * B_LOC]
        in_maps.append(
            {
                "xin": xp,
                "wqk": wqk,
                "wv": wv,
                "wp": wp,
                "bp": bp,
                "ebm": ebm,
                "ident": ident,
                "ones": ones,
                "onesr": onesr,
            }
        )
    return in_maps


def kernel(x, Wqkv, Wproj, bproj, sd_emb, sd, _trace=False):
    in_maps = _host_prep(x, Wqkv, Wproj, bproj, sd_emb, sd)
    args = _concat_inputs(in_maps)
    outs = _run_once(args)
    y = np.asarray(outs[0])  # [B, TOK, C] (cores concatenated on axis 0)
    return y.reshape(B, T, J, C).astype(np.float32)
